# revision 4
# baseline (speedup 1.0000x reference)
"""Fused self-attention (FCSelfAttention) Trainium2 Bass kernel.

Problem: X:[4,2048,512] fp32, W_qkv:[512,1536], W_out:[512,512], b_out:[512]
  qkv = X @ W_qkv ; q,k,v -> heads (B,H=8,N=2048,DH=64)
  scores[n,m] = k_n . q_m * DH**-0.5 ; softmax over m (query axis)
  out[n] = sum_e att[n,e] v[e] ; merge heads ; @ W_out + b_out

Sharding (8 cores): batch x head-group. Core c handles batch b=c//2 and
heads 4g..4g+3 where g=c%2 (data parallel over B=4, tensor parallel over
H=8 in halves). Each core computes a partial output projection for its
batch; the host sums the two partials per batch and adds b_out.

Device algorithm (per core), flash-style with scores kept transposed so
softmax reductions land on the TensorE contraction axis:
  S^T[m,n] = sum_d QT[d,m] KT[d,n]            (m = softmax axis, on partitions)
  P^T = exp(S^T * SCALE)                       (no max subtraction; |S|<9)
  PV:  lhsT = V_aug[e, 0:65] (col 64 = ones) -> psum[0:64]=out^T, psum[64]=Z
  A^T[d,n] = out^T[d,n] / Z[n] ; final = A^T.T @ W_out rows (per head, K=64)
"""

import sys

import numpy as np

_B, _N, _DIM = 4, 2048, 512
_H, _DH = 8, 64
_SCALE = _DH ** -0.5
_NCORES = 8
_HPC = 4              # heads per core
_HL = _HPC * _DH      # 256 local inner dim
_TC = _N // 128       # 16 token chunks
_KC = _DIM // 128     # 4 contraction chunks for projections

_cache = {}


def _emit(tc, xt, wq, wk, wv, wo, out, mybir):
    nc = tc.nc
    dt = mybir.dt
    f32, bf16 = dt.float32, dt.bfloat16
    Exp = mybir.ActivationFunctionType.Exp

    from contextlib import ExitStack

    with ExitStack() as ctx:
        weights = ctx.enter_context(tc.tile_pool(name="weights", bufs=1))
        xtp = ctx.enter_context(tc.tile_pool(name="xtp", bufs=1))
        qkp = ctx.enter_context(tc.tile_pool(name="qkp", bufs=1))
        vap = ctx.enter_context(tc.tile_pool(name="vap", bufs=1))
        atp = ctx.enter_context(tc.tile_pool(name="atp", bufs=1))
        ptp = ctx.enter_context(tc.tile_pool(name="ptp", bufs=3))
        zp = ctx.enter_context(tc.tile_pool(name="zp", bufs=2))
        zdp = ctx.enter_context(tc.tile_pool(name="zdp", bufs=2, space="DRAM"))
        outp = ctx.enter_context(tc.tile_pool(name="outp", bufs=3))
        psA = ctx.enter_context(tc.tile_pool(name="psA", bufs=2, space="PSUM"))
        psOp = ctx.enter_context(tc.tile_pool(name="psO", bufs=1, space="PSUM"))

        # ---- load inputs -------------------------------------------------
        xt_sb = []
        for kc in range(_KC):
            t = xtp.tile([128, _N], bf16, tag=f"xt{kc}", name=f"xt{kc}")
            nc.gpsimd.dma_start(t, xt[kc * 128:(kc + 1) * 128, :])
            xt_sb.append(t)

        wq_sb, wk_sb, wv_sb = [], [], []
        for name, dram, lst in (("wq", wq, wq_sb), ("wk", wk, wk_sb),
                                ("wv", wv, wv_sb)):
            for kc in range(_KC):
                t = weights.tile([128, _HL], bf16, tag=f"{name}{kc}", name=f"{name}{kc}")
                nc.gpsimd.dma_start(t, dram[kc * 128:(kc + 1) * 128, :])
                lst.append(t)
        wo_sb = []
        for h in range(_HPC):
            t = weights.tile([64, _DIM], bf16, tag=f"wo{h}", name=f"wo{h}")
            nc.gpsimd.dma_start(t, wo[h * 64:(h + 1) * 64, :])
            wo_sb.append(t)

        # ---- qkv projections --------------------------------------------
        # QT/KT: [hd, t] (2 chunks of 128 rows = 2 heads each)
        qt_sb, kt_sb = [], []
        for name, wsb, lst in (("qt", wq_sb, qt_sb), ("kt", wk_sb, kt_sb)):
            for hc in range(2):
                dst = qkp.tile([128, _N], bf16, tag=f"{name}{hc}", name=f"{name}{hc}")
                lst.append(dst)
                for tp in range(_N // 512):
                    ps = psA.tile([128, 512], f32, tag="mm")
                    for kc in range(_KC):
                        nc.tensor.matmul(
                            ps,
                            lhsT=wsb[kc][:, hc * 128:(hc + 1) * 128],
                            rhs=xt_sb[kc][:, tp * 512:(tp + 1) * 512],
                            start=(kc == 0), stop=(kc == _KC - 1),
                        )
                    nc.vector.tensor_copy(dst[:, tp * 512:(tp + 1) * 512], ps)

        # V augmented with a ones column: va[t][:, h, 0:64] = V, [..., 64] = 1
        va_sb = []
        for t in range(_TC):
            va = vap.tile([128, _HPC, 65], bf16, tag=f"va{t}", name=f"va{t}")
            va_sb.append(va)
            nc.gpsimd.memset(va[:, :, 64:65], 1.0)
            ps = psA.tile([128, _HL], f32, tag="mm")
            for kc in range(_KC):
                nc.tensor.matmul(
                    ps,
                    lhsT=xt_sb[kc][:, t * 128:(t + 1) * 128],
                    rhs=wv_sb[kc],
                    start=(kc == 0), stop=(kc == _KC - 1),
                )
            nc.vector.tensor_copy(
                va[:, :, 0:64], ps.rearrange("p (h d) -> p h d", h=_HPC))

        # ---- attention ---------------------------------------------------
        at_sb = []
        for h in range(_HPC):
            at_sb.append(atp.tile([64, _N], bf16, tag=f"at{h}", name=f"at{h}"))

        for h in range(_HPC):
            hc, hr = h // 2, (h % 2) * 64
            po = psOp.tile([65, _N], f32, tag="po")
            for ec in range(_TC):
                lhs_q = qt_sb[hc][hr:hr + 64, ec * 128:(ec + 1) * 128]
                for nh in range(2):
                    ps = psA.tile([128, 1024], f32, tag="mm")
                    for p2 in range(2):
                        ncol = nh * 1024 + p2 * 512
                        nc.tensor.matmul(
                            ps[:, p2 * 512:(p2 + 1) * 512],
                            lhsT=lhs_q,
                            rhs=kt_sb[hc][hr:hr + 64, ncol:ncol + 512],
                            start=True, stop=True,
                        )
                    pt = ptp.tile([128, 1024], bf16, tag="pt")
                    nc.scalar.activation(pt, ps, Exp, scale=_SCALE)
                    for p2 in range(2):
                        ncol = nh * 1024 + p2 * 512
                        nc.tensor.matmul(
                            po[0:65, ncol:ncol + 512],
                            lhsT=va_sb[ec][:, h, :],
                            rhs=pt[:, p2 * 512:(p2 + 1) * 512],
                            start=(ec == 0), stop=(ec == _TC - 1),
                        )
            # normalize: A^T = out^T * (1/Z) broadcast along partitions.
            # Copy psum->sbuf first (releases psO for the next head), take
            # 1/Z on DVE, bounce through DRAM to broadcast across partitions
            # (SBUF-source DMA cannot have partition stride 0).
            po_sb = zp.tile([65, _N], f32, tag="po_sb")
            nc.vector.tensor_copy(po_sb, po[0:65, :])
            nc.vector.reciprocal(po_sb[64:65, :], po_sb[64:65, :])
            zd = zdp.tile([1, _N], f32, tag="zd")
            nc.gpsimd.dma_start(zd, po_sb[64:65, :])
            zb = zp.tile([64, _N], f32, tag="zb")
            nc.gpsimd.dma_start(zb, zd.to_broadcast((64, _N)))
            nc.vector.tensor_mul(at_sb[h], po_sb[0:64, :], zb)

        # ---- output projection ------------------------------------------
        for t in range(_TC):
            ps = psA.tile([128, _DIM], f32, tag="mm")
            for h in range(_HPC):
                nc.tensor.matmul(
                    ps,
                    lhsT=at_sb[h][:, t * 128:(t + 1) * 128],
                    rhs=wo_sb[h],
                    start=(h == 0), stop=(h == _HPC - 1),
                )
            ot = outp.tile([128, _DIM], f32, tag="ot")
            nc.vector.tensor_copy(ot, ps)
            nc.gpsimd.dma_start(out[t * 128:(t + 1) * 128, :], ot)


def _build():
    if "/opt/trn_rl_repo" not in sys.path:
        sys.path.insert(0, "/opt/trn_rl_repo")
    from concourse import bacc, mybir
    import concourse.tile as tile

    dt = mybir.dt
    nc = bacc.Bacc("TRN2", target_bir_lowering=False, debug=False,
                   num_devices=_NCORES)
    xt = nc.dram_tensor("xt", [_DIM, _N], dt.bfloat16, kind="ExternalInput").ap()
    wq = nc.dram_tensor("wq", [_DIM, _HL], dt.bfloat16, kind="ExternalInput").ap()
    wk = nc.dram_tensor("wk", [_DIM, _HL], dt.bfloat16, kind="ExternalInput").ap()
    wv = nc.dram_tensor("wv", [_DIM, _HL], dt.bfloat16, kind="ExternalInput").ap()
    wo = nc.dram_tensor("wo", [_HL, _DIM], dt.bfloat16, kind="ExternalInput").ap()
    out = nc.dram_tensor("out", [_N, _DIM], dt.float32, kind="ExternalOutput").ap()

    with tile.TileContext(nc) as tc:
        _emit(tc, xt, wq, wk, wv, wo, out, mybir)
    nc.compile()
    return nc


def _get_nc():
    if "nc" not in _cache:
        _cache["nc"] = _build()
    return _cache["nc"]


def _shard_inputs(X, W_qkv, W_out):
    import ml_dtypes
    bf16 = ml_dtypes.bfloat16
    in_maps = []
    for c in range(_NCORES):
        b, g = c // 2, c % 2
        cols = slice(g * _HL, (g + 1) * _HL)
        in_maps.append({
            "xt": np.ascontiguousarray(X[b].T).astype(bf16),
            "wq": W_qkv[:, 0 * _DIM:][:, cols].astype(bf16),
            "wk": W_qkv[:, 1 * _DIM:][:, cols].astype(bf16),
            "wv": W_qkv[:, 2 * _DIM:][:, cols].astype(bf16),
            "wo": W_out[g * _HL:(g + 1) * _HL, :].astype(bf16),
        })
    return in_maps


def _run(inputs, trace=False):
    if "/opt/trn_rl_repo" not in sys.path:
        sys.path.insert(0, "/opt/trn_rl_repo")
    from concourse.bass_utils import run_bass_kernel_spmd

    X = np.asarray(inputs["X"], dtype=np.float32)
    W_qkv = np.asarray(inputs["W_qkv"], dtype=np.float32)
    W_out = np.asarray(inputs["W_out"], dtype=np.float32)
    b_out = np.asarray(inputs["b_out"], dtype=np.float32)

    nc = _get_nc()
    in_maps = _shard_inputs(X, W_qkv, W_out)
    res = run_bass_kernel_spmd(nc, in_maps, list(range(_NCORES)), trace=trace)

    out = np.empty((_B, _N, _DIM), dtype=np.float32)
    for b in range(_B):
        out[b] = res.results[2 * b]["out"] + res.results[2 * b + 1]["out"] + b_out
    return out, res.exec_time_ns


def kernel(**inputs) -> np.ndarray:
    out, _ = _run(inputs, trace=False)
    return out
